# revision 23
# baseline (speedup 1.0000x reference)
"""Trainium2 Bass kernel for the 4-layer sum/product circuit (segment_reduce).

Strategy: shard batch (4096) across 8 cores (512 each), zero communication.
Node-major layout: every layer tensor lives in DRAM as [n_nodes, 512] rows.
Gathers run as SWDGE dma_gather (single_packet=False; Q7 only generates
descriptors, the 16 SDMA engines move rows at HBM rate), 4096 indices per
call. Gathered row j lands on partition j%128, col j//128; indices are
pre-permuted host-side so the k legs of each output sit in one partition at
col l*S+s. Legs are summed with a pairwise tensor_tensor tree (2-byte
dtypes hit the DVE 2x mode; the final combine accumulates in f32), ACT
applies exp/ln, and an HWDGE dma_start writes result rows back node-major.
Log-domain tensors (xenc, l2) are fp16; exp-domain (e1, e3) are bf16 (f32
exponent range -- products can reach exp(-50)). Everything pipelines across
DMASW/DVE/ACT/DMAHW with multi-buffered tile pools.
"""

import math
import numpy as np
from contextlib import ExitStack

import concourse.bacc as bacc
import concourse.tile as tile
from concourse import bass, mybir
from concourse import library_config
from concourse.bass_utils import run_bass_kernel_spmd

N_CORES = 8
B = 4096
BPC = B // N_CORES          # 512 batch per core

N_XENC = 2050
N_L1 = 8192
N_L2 = 4096
N_L3 = 8192
N_OUT = 2048

FP = mybir.dt.float32
BF = mybir.dt.bfloat16
F16 = mybir.dt.float16
I16 = mybir.dt.int16

# (src, dst, n_src, n_out, k, ob, src_dt, dst_dt, act)
LAYERS = [
    ("xenc", "e1", N_XENC, N_L1, 4, 256, F16, BF, mybir.ActivationFunctionType.Exp),
    ("e1", "l2", N_L1, N_L2, 8, 128, BF, F16, mybir.ActivationFunctionType.Ln),
    ("l2", "e3", N_L2, N_L3, 4, 256, F16, BF, mybir.ActivationFunctionType.Exp),
    ("e3", "out", N_L3, N_OUT, 8, 128, BF, FP, mybir.ActivationFunctionType.Ln),
]


def _log1mexp(x):
    # match reference (Maechler 2012) in f32
    x = x.astype(np.float32)
    with np.errstate(divide="ignore", invalid="ignore", over="ignore"):
        a = np.log(-np.expm1(x)).astype(np.float32)
        b = np.log1p(-np.exp(x)).astype(np.float32)
    return np.where(x > -math.log(2.0), a, b).astype(np.float32)


def _build(nc):
    xenc_d = nc.dram_tensor("xenc", [N_XENC, BPC], F16, kind="ExternalInput").ap()
    e1_d = nc.dram_tensor("e1", [N_L1, BPC], BF, kind="Internal").ap()
    l2_d = nc.dram_tensor("l2", [N_L2, BPC], F16, kind="Internal").ap()
    e3_d = nc.dram_tensor("e3", [N_L3, BPC], BF, kind="Internal").ap()
    out_d = nc.dram_tensor("out", [N_OUT, BPC], FP, kind="ExternalOutput").ap()
    srcs = {"xenc": xenc_d, "e1": e1_d, "l2": l2_d, "e3": e3_d, "out": out_d}

    idx_d = {}
    for li, (_, _, _, n_out, k, _, _, _, _) in enumerate(LAYERS):
        idx_d[li] = nc.dram_tensor(f"idx{li}", [128, n_out * k // 16], I16,
                                   kind="ExternalInput").ap()

    with tile.TileContext(nc) as tc, ExitStack() as ctx:
        nc.gpsimd.load_library(library_config.mlp)
        idxp = ctx.enter_context(tc.tile_pool(name="idxp", bufs=8))
        gpool = ctx.enter_context(tc.tile_pool(name="gpool", bufs=12))
        tp4 = ctx.enter_context(tc.tile_pool(name="tp4", bufs=2))
        tp8 = ctx.enter_context(tc.tile_pool(name="tp8", bufs=4))
        apool = ctx.enter_context(tc.tile_pool(name="apool", bufs=2))
        rpool = ctx.enter_context(tc.tile_pool(name="rpool", bufs=2))

        for li, (sname, dname, n_src, n_out, k, ob, sdt, ddt, act) in enumerate(LAYERS):
            src_ap = srcs[sname]
            dst_ap = srcs[dname]
            ni = ob * k                      # idxs per call
            S = ob // 128                    # out-slots per partition
            C = ni // 128                    # gather cols per partition
            cols16 = ni // 16                # idx cols per call
            n_calls = n_out // ob

            rbytes = S * BPC * mybir.dt.size(ddt)
            for ci in range(n_calls):
                it = idxp.tile([128, cols16], I16, tag="idx")
                ieng = nc.scalar if ci % 2 == 0 else nc.sync
                ieng.dma_start(
                    it[:], idx_d[li][:, ci * cols16:(ci + 1) * cols16])
                g = gpool.tile([128, C, BPC], sdt, tag="g")
                nc.gpsimd.dma_gather(
                    g[:], src_ap[:], it[:],
                    ni, ni, BPC, single_packet=False,
                    queue_num=ci % 4,
                )
                # pairwise leg-sum tree: legs at cols [l*S, (l+1)*S)
                cur = [g[:, l * S:(l + 1) * S, :] for l in range(k)]
                lvl = 0
                while len(cur) > 2:
                    nxt = []
                    for i in range(0, len(cur), 2):
                        tp = tp4 if k == 4 else tp8
                        t = tp.tile([128, S, BPC], sdt, tag=f"t{k}_{lvl}")
                        nc.vector.tensor_tensor(t[:], cur[i], cur[i + 1],
                                                mybir.AluOpType.add)
                        nxt.append(t[:])
                    cur = nxt
                    lvl += 1
                acc = apool.tile([128, S, BPC], FP, tag=f"a{k}")
                nc.vector.tensor_tensor(acc[:], cur[0], cur[1],
                                        mybir.AluOpType.add)
                res = rpool.tile([128, S, BPC], ddt, tag=f"r{rbytes}")
                nc.scalar.activation(res[:], acc[:], act)
                dst = dst_ap[ci * ob:(ci + 1) * ob].rearrange(
                    "(s p) e -> p s e", p=128)
                weng = nc.sync if ci % 2 == 0 else nc.scalar
                weng.dma_start(dst, res[:])
    nc.compile()
    return nc


def _perm_idx(idxl: np.ndarray, ob: int) -> np.ndarray:
    """[n_out, k] -> wrapped int16 [128, n_out*k/16] in dma_gather order.

    Within a call of `ob` outputs: gathered row j -> out[p=j%128, c=j//128],
    col c = l*S + s so legs of output o = base + s*128 + p sit in one
    partition as contiguous [S, BPC] slices per leg.
    """
    n_out, k = idxl.shape
    S = ob // 128
    n_calls = n_out // ob
    o = np.arange(n_out).reshape(n_calls, S, 128)       # [ci, s, p]
    a = idxl[o]                                         # [ci, S, 128, k]
    a = a.transpose(0, 3, 1, 2)                         # [ci, l, S, p]
    flat = a.reshape(-1)                                # j = ((ci*k+l)*S+s)*128+p
    w = flat.reshape(-1, 16).T.astype(np.int16)         # [16, Q/16]
    return np.tile(w, (8, 1))


_CACHED_NC = None
_LAST_IN_MAPS = None


def kernel(pos, idx0, idx1, idx2, idx3):
    global _CACHED_NC, _LAST_IN_MAPS
    pos = np.asarray(pos, dtype=np.float32)

    # host-side input encoding: x_enc [2050, 4096] log-probs
    neg = _log1mexp(pos)
    n, b = pos.shape
    xenc = np.zeros((2 * n + 2, b), np.float32)
    xenc[1] = 0.0
    xenc[2::2] = pos
    xenc[3::2] = neg
    # row 0 is -inf in the reference but never gathered (idx0 >= 1); keep 0.
    xenc16 = xenc.astype(np.float16)

    idx_maps = {}
    for li, idxl in enumerate((idx0, idx1, idx2, idx3)):
        ob = LAYERS[li][5]
        idx_maps[f"idx{li}"] = _perm_idx(np.asarray(idxl, dtype=np.int64), ob)

    if _CACHED_NC is None:
        _CACHED_NC = _build(bacc.Bacc("TRN2", target_bir_lowering=False,
                                      debug=False, num_swdge_queues=4))
    nc = _CACHED_NC

    in_maps = []
    for c in range(N_CORES):
        in_maps.append({
            "xenc": np.ascontiguousarray(xenc16[:, c * BPC:(c + 1) * BPC]),
            **idx_maps,
        })
    _LAST_IN_MAPS = in_maps
    res = run_bass_kernel_spmd(nc, in_maps, list(range(N_CORES)))
    out = np.empty((N_OUT, B), np.float32)
    for c in range(N_CORES):
        out[:, c * BPC:(c + 1) * BPC] = res.results[c]["out"]
    return out


# revision 31
# speedup vs baseline: 1.0654x; 1.0654x over previous
"""Trainium2 Bass kernel for the 4-layer sum/product circuit (segment_reduce).

Strategy: shard batch (4096) across 8 cores (512 each), zero communication.
Node-major layout: every layer tensor lives in DRAM as [n_nodes, 512] rows.
Gathers run as SWDGE dma_gather (single_packet=False; Q7 only generates
descriptors, the 16 SDMA engines move rows at HBM rate), 4096 indices per
call. Gathered row j lands on partition j%128, col j//128; indices are
pre-permuted host-side so the k legs of each output sit in one partition at
col l*S+s. Legs are summed with a pairwise tensor_tensor tree (2-byte
dtypes hit the DVE 2x mode; the final combine accumulates in f32), ACT
applies exp/ln, and an HWDGE dma_start writes result rows back node-major.
Log-domain tensors (xenc, l2) are fp16; exp-domain (e1, e3) are bf16 (f32
exponent range -- products can reach exp(-50)). Everything pipelines across
DMASW/DVE/ACT/DMAHW with multi-buffered tile pools.
"""

import math
import numpy as np
from contextlib import ExitStack

import concourse.bacc as bacc
import concourse.tile as tile
from concourse import bass, mybir
from concourse import library_config
from concourse.bass_utils import run_bass_kernel_spmd

N_CORES = 8
B = 4096
BPC = B // N_CORES          # 512 batch per core

N_XENC = 2050
N_L1 = 8192
N_L2 = 4096
N_L3 = 8192
N_OUT = 2048

FP = mybir.dt.float32
BF = mybir.dt.bfloat16
F16 = mybir.dt.float16
I16 = mybir.dt.int16

# (src, dst, n_src, n_out, k, ob, src_dt, dst_dt, act)
LAYERS = [
    ("xenc", "e1", N_XENC, N_L1, 4, 256, F16, BF, mybir.ActivationFunctionType.Exp),
    ("e1", "l2", N_L1, N_L2, 8, 128, BF, F16, mybir.ActivationFunctionType.Ln),
    ("l2", "e3", N_L2, N_L3, 4, 256, F16, BF, mybir.ActivationFunctionType.Exp),
    ("e3", "out", N_L3, N_OUT, 8, 128, BF, FP, mybir.ActivationFunctionType.Ln),
]


def _log1mexp(x):
    # match reference (Maechler 2012) in f32
    x = x.astype(np.float32)
    with np.errstate(divide="ignore", invalid="ignore", over="ignore"):
        a = np.log(-np.expm1(x)).astype(np.float32)
        b = np.log1p(-np.exp(x)).astype(np.float32)
    return np.where(x > -math.log(2.0), a, b).astype(np.float32)


def _build(nc):
    xenc_d = nc.dram_tensor("xenc", [N_XENC, BPC], F16, kind="ExternalInput").ap()
    e1_d = nc.dram_tensor("e1", [N_L1, BPC], BF, kind="Internal").ap()
    l2_d = nc.dram_tensor("l2", [N_L2, BPC], F16, kind="Internal").ap()
    e3_d = nc.dram_tensor("e3", [N_L3, BPC], BF, kind="Internal").ap()
    out_d = nc.dram_tensor("out", [N_OUT, BPC], FP, kind="ExternalOutput").ap()
    srcs = {"xenc": xenc_d, "e1": e1_d, "l2": l2_d, "e3": e3_d, "out": out_d}

    idx_d = {}
    for li, (_, _, _, n_out, k, _, _, _, _) in enumerate(LAYERS):
        idx_d[li] = nc.dram_tensor(f"idx{li}", [128, n_out * k // 16], I16,
                                   kind="ExternalInput").ap()

    with tile.TileContext(nc) as tc, ExitStack() as ctx:
        nc.gpsimd.load_library(library_config.mlp)
        idxp = ctx.enter_context(tc.tile_pool(name="idxp", bufs=8))
        gpool = ctx.enter_context(tc.tile_pool(name="gpool", bufs=8))
        tp4 = ctx.enter_context(tc.tile_pool(name="tp4", bufs=2))
        tp8 = ctx.enter_context(tc.tile_pool(name="tp8", bufs=4))
        apool = ctx.enter_context(tc.tile_pool(name="apool", bufs=2))
        rpool = ctx.enter_context(tc.tile_pool(name="rpool", bufs=2))

        for li, (sname, dname, n_src, n_out, k, ob, sdt, ddt, act) in enumerate(LAYERS):
            src_ap = srcs[sname]
            dst_ap = srcs[dname]
            ni = ob * k                      # idxs per call
            S = ob // 128                    # out-slots per partition
            C = ni // 128                    # gather cols per partition
            cols16 = ni // 16                # idx cols per call
            n_calls = n_out // ob

            rbytes = S * BPC * mybir.dt.size(ddt)
            for ci in range(n_calls):
                it = idxp.tile([128, cols16], I16, tag="idx")
                ieng = nc.scalar if ci % 2 == 0 else nc.sync
                ieng.dma_start(
                    it[:], idx_d[li][:, ci * cols16:(ci + 1) * cols16])
                g = gpool.tile([128, C, BPC], sdt, tag="g")
                nc.gpsimd.dma_gather(
                    g[:], src_ap[:], it[:],
                    ni, ni, BPC, single_packet=False,
                    queue_num=ci % 4,
                )
                # pairwise leg-sum tree: legs at cols [l*S, (l+1)*S)
                cur = [g[:, l * S:(l + 1) * S, :] for l in range(k)]
                lvl = 0
                while len(cur) > 2:
                    nxt = []
                    for i in range(0, len(cur), 2):
                        tp = tp4 if k == 4 else tp8
                        t = tp.tile([128, S, BPC], sdt, tag=f"t{k}_{lvl}")
                        nc.vector.tensor_tensor(t[:], cur[i], cur[i + 1],
                                                mybir.AluOpType.add)
                        nxt.append(t[:])
                    cur = nxt
                    lvl += 1
                acc = apool.tile([128, S, BPC], FP, tag=f"a{k}")
                nc.vector.tensor_tensor(acc[:], cur[0], cur[1],
                                        mybir.AluOpType.add)
                res = rpool.tile([128, S, BPC], ddt, tag=f"r{rbytes}")
                nc.scalar.activation(res[:], acc[:], act)
                dst = dst_ap[ci * ob:(ci + 1) * ob].rearrange(
                    "(s p) e -> p s e", p=128)
                weng = nc.sync if ci % 2 == 0 else nc.scalar
                weng.dma_start(dst, res[:])
    nc.compile()
    return nc


def _perm_idx(idxl: np.ndarray, ob: int) -> np.ndarray:
    """[n_out, k] -> wrapped int16 [128, n_out*k/16] in dma_gather order.

    Within a call of `ob` outputs: gathered row j -> out[p=j%128, c=j//128],
    col c = l*S + s so legs of output o = base + s*128 + p sit in one
    partition as contiguous [S, BPC] slices per leg.
    """
    n_out, k = idxl.shape
    S = ob // 128
    n_calls = n_out // ob
    o = np.arange(n_out).reshape(n_calls, S, 128)       # [ci, s, p]
    a = idxl[o]                                         # [ci, S, 128, k]
    a = a.transpose(0, 3, 1, 2)                         # [ci, l, S, p]
    flat = a.reshape(-1)                                # j = ((ci*k+l)*S+s)*128+p
    w = flat.reshape(-1, 16).T.astype(np.int16)         # [16, Q/16]
    return np.tile(w, (8, 1))


_CACHED_NC = None
_LAST_IN_MAPS = None


def kernel(pos, idx0, idx1, idx2, idx3):
    global _CACHED_NC, _LAST_IN_MAPS
    pos = np.asarray(pos, dtype=np.float32)

    # host-side input encoding: x_enc [2050, 4096] log-probs
    neg = _log1mexp(pos)
    n, b = pos.shape
    xenc = np.zeros((2 * n + 2, b), np.float32)
    xenc[1] = 0.0
    xenc[2::2] = pos
    xenc[3::2] = neg
    # row 0 is -inf in the reference but never gathered (idx0 >= 1); keep 0.
    xenc16 = xenc.astype(np.float16)

    idx_maps = {}
    for li, idxl in enumerate((idx0, idx1, idx2, idx3)):
        ob = LAYERS[li][5]
        idx_maps[f"idx{li}"] = _perm_idx(np.asarray(idxl, dtype=np.int64), ob)

    if _CACHED_NC is None:
        _CACHED_NC = _build(bacc.Bacc("TRN2", target_bir_lowering=False,
                                      debug=False, num_swdge_queues=4))
    nc = _CACHED_NC

    in_maps = []
    for c in range(N_CORES):
        in_maps.append({
            "xenc": np.ascontiguousarray(xenc16[:, c * BPC:(c + 1) * BPC]),
            **idx_maps,
        })
    _LAST_IN_MAPS = in_maps
    res = run_bass_kernel_spmd(nc, in_maps, list(range(N_CORES)))
    out = np.empty((N_OUT, B), np.float32)
    for c in range(N_CORES):
        out[:, c * BPC:(c + 1) * BPC] = res.results[c]["out"]
    return out
